# revision 43
# baseline (speedup 1.0000x reference)
"""EpsSupInfoNCE loss on 8 Trainium2 NeuronCores.

Math (reference): logits = (E @ E.T)/temp;  same[i,j] = (label_i == label_j)
  S_j   = sum_i exp(logits[i,j]) * (1 - same[i,j])     (masked column sums)
  ce_ij = log(exp(-eps) + S_j * exp(-logits[i,j]))     for same-label i != j
  loss  = sum_j (1/count_j) * sum_i ce_ij / B

Strategy: columns are sharded over 8 cores; the HOST sorts all columns by
label first. Core c owns 1024 sorted columns whose same-label rows then
live in ONE contiguous row interval, padded to a 2048-row "window" (max
class size ~130, so 1024 + 2*130 always fits). Rows are passed to each
core pre-split into et_win [D,2048] and et_main [D,6144] (order of rows
is irrelevant for the sums), which keeps the program SPMD-identical
across cores while the actual window offset varies per core.

Per 128-column tile:
  main rows:   logits matmul (fp32r, single PE pass) -> PSUM;
               ACT exp(+l/temp) with fused accum_out -> S partials.
               No label mask needed: no same-label pairs here.
  window rows: logits matmul + one-hot mask matmul (-4.5 -> -C in logit
               units) -> PSUM; ACT exp(l/temp - C*same) -> Pwin, whose
               fused accum IS the masked S_win partial; DVE reciprocal
               R = 1/Pwin; ACT Ln(m_j * R + 1) with accum_out -> A_j
               partials, m_j = S_j*e^(eps-C): equals ce+eps at same-label
               entries, ~1e-17 at different-label ones.
Host: numer_sum_j = A_j - eps*count_j - log1p(S_j e^(eps-l_jj)) (the
diagonal term, which carries its own +eps), then a tiny f64 reduction. Out-of-window same-label
terms do not exist; out-of-window Ln terms are < 1e-13 and are dropped.
"""
import numpy as np
import ml_dtypes
from contextlib import ExitStack

import concourse.bacc as bacc
import concourse.tile as tile
from concourse import mybir
from concourse.bass_utils import run_bass_kernel_spmd

B, D = 8192, 128
NCLS = 100
NCORES = 8
COLS = B // NCORES            # 1024 columns per core
NCT = COLS // 128             # 8 col-tiles per core
WIN = 2048                    # window rows per core
MAIN = B - WIN                # 6144 main rows per core
GROUP = 2048                  # rows per PSUM group (4 banks)
NGM = MAIN // GROUP           # 3 main groups
NSUB = GROUP // 512

TEMP = 0.07
EPS = 0.25
SCALE = float(np.float32(1.0) / np.float32(TEMP))   # exp scale (fp32 value)
MASKVAL = -4.5                                      # bf16-exact additive mask
C_USED = 4.5 * SCALE                                # mask size in logit units
MCONST = float(np.exp(EPS - C_USED))                # e^(eps-C)

_cache = {}


def _patch_act_tables():
    """Steer Exp and Ln onto the one table set holding both, so Exp/Ln
    alternation doesn't thrash ACT_TABLE_LOADs. Set ids are indices into
    act_info.json, so keep dict length/order and just hide exp/ln
    elsewhere."""
    import concourse.hw_specs as hw_specs
    from concourse import mybir as _mb
    if getattr(bacc, "_act_tables_patched", False):
        return
    orig = hw_specs.get_activation_tables

    def steer(arch):
        t = orig(arch)
        exp, ln = (_mb.ActivationFunctionType.Exp, _mb.ActivationFunctionType.Ln)
        if "natural_log_exp_and_others" not in t:
            return t
        return {k: (fns if k == "natural_log_exp_and_others"
                    else fns - {exp, ln}) for k, fns in t.items()}

    bacc.get_activation_tables = steer
    bacc._act_tables_patched = True


def _build():
    dt = mybir.dt
    _patch_act_tables()
    nc = bacc.Bacc("TRN2", target_bir_lowering=False, debug=False,
                   num_devices=NCORES)
    et_main = nc.dram_tensor("et_main", [D, MAIN], dt.float32,
                             kind="ExternalInput").ap()
    et_win = nc.dram_tensor("et_win", [D, WIN], dt.float32,
                            kind="ExternalInput").ap()
    et_own = nc.dram_tensor("et_own", [D, COLS], dt.float32,
                            kind="ExternalInput").ap()
    oh_win = nc.dram_tensor("oh_win", [NCLS, WIN], dt.bfloat16,
                            kind="ExternalInput").ap()
    ohn_own = nc.dram_tensor("ohn_own", [NCLS, COLS], dt.bfloat16,
                             kind="ExternalInput").ap()
    NSLOT = NGM + 1                                    # S slots per col-tile
    out = nc.dram_tensor("out", [128, NCT * NSLOT + NCT], dt.float32,
                         kind="ExternalOutput").ap()

    with tile.TileContext(nc) as tc:
        with ExitStack() as ctx:
            const_pool = ctx.enter_context(tc.tile_pool(name="consts", bufs=1))
            p_pool = ctx.enter_context(tc.tile_pool(name="pwin", bufs=2))
            r_pool = ctx.enter_context(tc.tile_pool(name="rbuf", bufs=2))
            d_pool = ctx.enter_context(tc.tile_pool(name="dump", bufs=2))
            stage_pool = ctx.enter_context(tc.tile_pool(name="stage", bufs=2))
            ps_pool = ctx.enter_context(
                tc.tile_pool(name="psum", bufs=2, space="PSUM"))

            # fp32r matmul operands must be rounded by a compute op: DMA to
            # fp32 staging, DVE-copy (rounds) into fp32r tiles. Load order:
            # what the first col-tile needs first.
            # Load order mirrors first-consumption order: ct0's window work
            # needs et_own[:, :128], et_win, oh_win, ohn_own; the main
            # groups then need et_main chunks. Two DMA queues in parallel.
            # ct0's first matmul needs only et_own[:, :128] and et_win chunk
            # 0, so chunk those finely and alternate DMA queues to overlap
            # transfer with the first window matmuls.
            t_et_own = const_pool.tile([D, COLS], dt.float32r)
            st0 = stage_pool.tile([D, COLS], dt.float32, tag="st_own")
            nc.sync.dma_start(st0[:, 0:512], et_own[:, 0:512])
            nc.vector.tensor_copy(t_et_own[:, 0:512], st0[:, 0:512])
            t_et_win = const_pool.tile([D, WIN], dt.float32r)
            stw = stage_pool.tile([D, GROUP], dt.float32, tag="st_win")
            nc.gpsimd.dma_start(stw[:], et_win[:])
            for h in range(2):
                nc.vector.tensor_copy(t_et_win[:, h * 1024:(h + 1) * 1024],
                                      stw[:, h * 1024:(h + 1) * 1024])
            t_oh_win = const_pool.tile([NCLS, WIN], dt.bfloat16)
            nc.sync.dma_start(t_oh_win[:], oh_win[:])
            t_ohn_own = const_pool.tile([NCLS, COLS], dt.bfloat16)
            nc.sync.dma_start(t_ohn_own[:], ohn_own[:])
            nc.sync.dma_start(st0[:, 512:], et_own[:, 512:])
            nc.vector.tensor_copy(t_et_own[:, 512:], st0[:, 512:])
            t_et_main = const_pool.tile([D, MAIN], dt.float32r)
            for i in range(NGM):
                st = stage_pool.tile([D, GROUP], dt.float32, tag="st")
                nc.sync.dma_start(st[:], et_main[:, i * GROUP:(i + 1) * GROUP])
                for h in range(2):
                    lo = i * GROUP + h * 1024
                    nc.vector.tensor_copy(t_et_main[:, lo:lo + 1024],
                                          st[:, h * 1024:(h + 1) * 1024])

            # Warm the PE's HAM clock gate during the input-DMA wait:
            # ~14 dummy matmuls on zeroed tiles (~4-5us of PE activity)
            # flip the clock from 1.2 to 2.4 GHz before real work arrives.
            zl = const_pool.tile([NCLS, 128], dt.bfloat16)
            zr = const_pool.tile([NCLS, 512], dt.bfloat16)
            nc.gpsimd.memset(zl[:], 0.0)
            nc.gpsimd.memset(zr[:], 0.0)
            warm_ps = ps_pool.tile([128, 512], dt.float32, tag="ps")
            for _ in range(14):
                nc.tensor.matmul(warm_ps[:], zl[:], zr[:],
                                 start=True, stop=True)

            s_part = const_pool.tile([128, NCT * NSLOT], dt.float32)
            a_part = const_pool.tile([128, NCT], dt.float32)
            m_raw = const_pool.tile([128, NCT], dt.float32)
            m_sb = const_pool.tile([128, NCT], dt.float32)

            def emit_ln(ct, R):
                # ce-sum: Ln(m_j / Pwin + 1), fused per-column accumulate.
                dump = d_pool.tile([128, WIN], dt.float32, tag="dump")
                nc.scalar.activation(
                    dump[:], R[:], mybir.ActivationFunctionType.Ln,
                    scale=m_sb[:, ct:ct + 1], bias=1.0,
                    accum_out=a_part[:, ct:ct + 1])

            prev = None          # (ct, P) whose Ln is deferred one col-tile
            for ct in range(NCT):
                lhs_et = t_et_own[:, ct * 128:(ct + 1) * 128]
                lhs_oh = t_ohn_own[:, ct * 128:(ct + 1) * 128]

                # ---- window rows first: masked; feed S and the ce sum.
                # Doing these before the main groups starts the DVE S/m
                # chain early so the deferred Ln is never waiting.
                ps = ps_pool.tile([128, GROUP], dt.float32, tag="ps")
                for n in range(NSUB):
                    nc.tensor.matmul(
                        ps[:, n * 512:(n + 1) * 512], lhs_et,
                        t_et_win[:, n * 512:(n + 1) * 512],
                        start=True, stop=False)
                for n in range(NSUB):
                    nc.tensor.matmul(
                        ps[:, n * 512:(n + 1) * 512], lhs_oh,
                        t_oh_win[:, n * 512:(n + 1) * 512],
                        start=False, stop=True)
                # Pwin = exp(l - C*same); its fused accum IS the masked S_win.
                # The Ln input exp(-l + C*same) comes from the DVE reciprocal.
                P = p_pool.tile([128, WIN], dt.float32, tag="P")
                slot = ct * NSLOT + NGM
                nc.scalar.activation(
                    P[:], ps[:], mybir.ActivationFunctionType.Exp,
                    scale=SCALE, accum_out=s_part[:, slot:slot + 1])
                R = r_pool.tile([128, WIN], dt.float32, tag="R")
                nc.vector.reciprocal_approx_fast(out=R[:], in_=P[:])

                # ---- main rows: unmasked, only feed S ----
                for g in range(NGM):
                    r0 = g * GROUP
                    ps = ps_pool.tile([128, GROUP], dt.float32, tag="ps")
                    for n in range(NSUB):
                        nc.tensor.matmul(
                            ps[:, n * 512:(n + 1) * 512], lhs_et,
                            t_et_main[:, r0 + n * 512:r0 + (n + 1) * 512],
                            start=True, stop=True)
                    dump = d_pool.tile([128, GROUP], dt.float32, tag="dump")
                    slot = ct * NSLOT + g
                    nc.scalar.activation(
                        dump[:], ps[:], mybir.ActivationFunctionType.Exp,
                        scale=SCALE, accum_out=s_part[:, slot:slot + 1])

                nc.vector.reduce_sum(
                    m_raw[:, ct:ct + 1],
                    s_part[:, ct * NSLOT:(ct + 1) * NSLOT],
                    axis=mybir.AxisListType.X)
                nc.vector.tensor_scalar_mul(
                    m_sb[:, ct:ct + 1], m_raw[:, ct:ct + 1], MCONST)

                # Defer this tile's Ln so the ACT FIFO can run the next
                # tile's exps while the DVE S/m chain completes.
                if prev is not None:
                    emit_ln(*prev)
                prev = (ct, R)
            emit_ln(*prev)

            nc.sync.dma_start(out[:, 0:NCT * NSLOT], s_part[:])
            nc.sync.dma_start(out[:, NCT * NSLOT:], a_part[:])
    nc.compile()
    return nc


def _get_nc():
    if "nc" not in _cache:
        _cache["nc"] = _build()
    return _cache["nc"]


def _prepare(embeds, labels):
    embeds = np.ascontiguousarray(np.asarray(embeds, dtype=np.float32))
    labels_i = np.asarray(labels).astype(np.int64)
    assert embeds.shape == (B, D)

    # Sort columns (and rows -- it is the same axis) by label so each
    # core's same-label rows are contiguous.
    perm = np.argsort(labels_i, kind="stable")
    lab = labels_i[perm]
    emb = embeds[perm]

    et = np.ascontiguousarray(emb.T)                      # [D, B] f32
    oh = np.zeros((NCLS, B), dtype=ml_dtypes.bfloat16)
    oh[lab, np.arange(B)] = ml_dtypes.bfloat16(1.0)
    ohn = (oh.astype(np.float32) * np.float32(MASKVAL)).astype(ml_dtypes.bfloat16)

    # class start/end in sorted order
    starts = np.searchsorted(lab, np.arange(NCLS), side="left")
    ends = np.searchsorted(lab, np.arange(NCLS), side="right")

    in_maps = []
    windows = []
    for c in range(NCORES):
        lo, hi = c * COLS, (c + 1) * COLS
        r_lo = int(starts[lab[lo]])
        r_hi = int(ends[lab[hi - 1]])
        assert r_hi - r_lo <= WIN, f"window overflow: {r_hi - r_lo}"
        w0 = max(0, min((r_lo // 512) * 512, B - WIN))
        if w0 + WIN < r_hi:                       # need to shift right
            w0 = min(((r_hi + 511) // 512) * 512 - WIN, B - WIN)
        assert w0 <= r_lo and w0 + WIN >= r_hi
        windows.append(w0)
        main_idx = np.r_[0:w0, w0 + WIN:B]
        in_maps.append({
            "et_main": np.ascontiguousarray(et[:, main_idx]),
            "et_win": np.ascontiguousarray(et[:, w0:w0 + WIN]),
            "et_own": np.ascontiguousarray(et[:, lo:hi]),
            "oh_win": np.ascontiguousarray(oh[:, w0:w0 + WIN]),
            "ohn_own": np.ascontiguousarray(ohn[:, lo:hi]),
        })
    return in_maps, lab, emb


def _combine(results, lab, emb):
    NSLOT = NGM + 1
    S = np.empty(B, dtype=np.float64)
    A = np.empty(B, dtype=np.float64)
    for c in range(NCORES):
        o = results[c]["out"].astype(np.float64)
        s = o[:, :NCT * NSLOT].reshape(128, NCT, NSLOT).sum(-1)   # [p, ct]
        a = o[:, NCT * NSLOT:NCT * NSLOT + NCT]                   # [p, ct]
        S[c * COLS:(c + 1) * COLS] = s.T.reshape(-1)              # j = ct*128+p
        A[c * COLS:(c + 1) * COLS] = a.T.reshape(-1)

    counts = np.bincount(lab, minlength=NCLS)
    count_j = counts[lab].astype(np.float64) - 1.0
    l_jj = (emb.astype(np.float64) ** 2).sum(1) * SCALE
    # A_j = sum_{in_numer}(ce+eps) + (ce_jj+eps); u_jj = ce_jj + eps, so
    # numer = A_j - eps*count_j - u_jj.
    u_jj = np.log1p(S * np.exp(EPS - l_jj))
    numer = A - EPS * count_j - u_jj
    loss = (numer / count_j).sum() / B
    return np.asarray(loss, dtype=np.float32)


def kernel(embeds, labels):
    in_maps, lab, emb = _prepare(embeds, labels)
    nc = _get_nc()
    res = run_bass_kernel_spmd(nc, in_maps, list(range(NCORES)))
    return _combine(res.results, lab, emb)


# revision 44
# speedup vs baseline: 1.0048x; 1.0048x over previous
"""EpsSupInfoNCE loss on 8 Trainium2 NeuronCores.

Math (reference): logits = (E @ E.T)/temp;  same[i,j] = (label_i == label_j)
  S_j   = sum_i exp(logits[i,j]) * (1 - same[i,j])     (masked column sums)
  ce_ij = log(exp(-eps) + S_j * exp(-logits[i,j]))     for same-label i != j
  loss  = sum_j (1/count_j) * sum_i ce_ij / B

Strategy: columns are sharded over 8 cores; the HOST sorts all columns by
label first. Core c owns 1024 sorted columns whose same-label rows then
live in ONE contiguous row interval, padded to a 2048-row "window" (max
class size ~130, so 1024 + 2*130 always fits). Rows are passed to each
core pre-split into et_win [D,2048] and et_main [D,6144] (order of rows
is irrelevant for the sums), which keeps the program SPMD-identical
across cores while the actual window offset varies per core.

Per 128-column tile:
  main rows:   logits matmul (fp32r, single PE pass) -> PSUM;
               ACT exp(+l/temp) with fused accum_out -> S partials.
               No label mask needed: no same-label pairs here.
  window rows: logits matmul + one-hot mask matmul (-4.5 -> -C in logit
               units) -> PSUM; ACT exp(l/temp - C*same) -> Pwin, whose
               fused accum IS the masked S_win partial; DVE reciprocal
               R = 1/Pwin; ACT Ln(m_j * R + 1) with accum_out -> A_j
               partials, m_j = S_j*e^(eps-C): equals ce+eps at same-label
               entries, ~1e-17 at different-label ones.
Host: numer_sum_j = A_j - eps*count_j - log1p(S_j e^(eps-l_jj)) (the
diagonal term, which carries its own +eps), then a tiny f64 reduction. Out-of-window same-label
terms do not exist; out-of-window Ln terms are < 1e-13 and are dropped.
"""
import numpy as np
import ml_dtypes
from contextlib import ExitStack

import concourse.bacc as bacc
import concourse.tile as tile
from concourse import mybir
from concourse.bass_utils import run_bass_kernel_spmd

B, D = 8192, 128
NCLS = 100
NCORES = 8
COLS = B // NCORES            # 1024 columns per core
NCT = COLS // 128             # 8 col-tiles per core
WIN = 2048                    # window rows per core
MAIN = B - WIN                # 6144 main rows per core
GROUP = 2048                  # rows per PSUM group (4 banks)
NGM = MAIN // GROUP           # 3 main groups
NSUB = GROUP // 512

TEMP = 0.07
EPS = 0.25
SCALE = float(np.float32(1.0) / np.float32(TEMP))   # exp scale (fp32 value)
MASKVAL = -4.5                                      # bf16-exact additive mask
C_USED = 4.5 * SCALE                                # mask size in logit units
MCONST = float(np.exp(EPS - C_USED))                # e^(eps-C)

_cache = {}


def _patch_act_tables():
    """Steer Exp and Ln onto the one table set holding both, so Exp/Ln
    alternation doesn't thrash ACT_TABLE_LOADs. Set ids are indices into
    act_info.json, so keep dict length/order and just hide exp/ln
    elsewhere."""
    import concourse.hw_specs as hw_specs
    from concourse import mybir as _mb
    if getattr(bacc, "_act_tables_patched", False):
        return
    orig = hw_specs.get_activation_tables

    def steer(arch):
        t = orig(arch)
        exp, ln = (_mb.ActivationFunctionType.Exp, _mb.ActivationFunctionType.Ln)
        if "natural_log_exp_and_others" not in t:
            return t
        return {k: (fns if k == "natural_log_exp_and_others"
                    else fns - {exp, ln}) for k, fns in t.items()}

    bacc.get_activation_tables = steer
    bacc._act_tables_patched = True


def _build():
    dt = mybir.dt
    _patch_act_tables()
    nc = bacc.Bacc("TRN2", target_bir_lowering=False, debug=False,
                   num_devices=NCORES)
    et_main = nc.dram_tensor("et_main", [D, MAIN], dt.float32,
                             kind="ExternalInput").ap()
    et_win = nc.dram_tensor("et_win", [D, WIN], dt.float32,
                            kind="ExternalInput").ap()
    et_own = nc.dram_tensor("et_own", [D, COLS], dt.float32,
                            kind="ExternalInput").ap()
    oh_win = nc.dram_tensor("oh_win", [NCLS, WIN], dt.bfloat16,
                            kind="ExternalInput").ap()
    ohn_own = nc.dram_tensor("ohn_own", [NCLS, COLS], dt.bfloat16,
                             kind="ExternalInput").ap()
    NSLOT = NGM + 1                                    # S slots per col-tile
    out = nc.dram_tensor("out", [128, NCT * NSLOT + NCT], dt.float32,
                         kind="ExternalOutput").ap()

    with tile.TileContext(nc) as tc:
        with ExitStack() as ctx:
            const_pool = ctx.enter_context(tc.tile_pool(name="consts", bufs=1))
            p_pool = ctx.enter_context(tc.tile_pool(name="pwin", bufs=2))
            r_pool = ctx.enter_context(tc.tile_pool(name="rbuf", bufs=2))
            d_pool = ctx.enter_context(tc.tile_pool(name="dump", bufs=2))
            stage_pool = ctx.enter_context(tc.tile_pool(name="stage", bufs=2))
            ps_pool = ctx.enter_context(
                tc.tile_pool(name="psum", bufs=2, space="PSUM"))

            # fp32r matmul operands must be rounded by a compute op: DMA to
            # fp32 staging, DVE-copy (rounds) into fp32r tiles. Load order:
            # what the first col-tile needs first.
            # Load order mirrors first-consumption order: ct0's window work
            # needs et_own[:, :128], et_win, oh_win, ohn_own; the main
            # groups then need et_main chunks. Two DMA queues in parallel.
            # ct0's first matmul needs only et_own[:, :128] and et_win chunk
            # 0, so chunk those finely and alternate DMA queues to overlap
            # transfer with the first window matmuls.
            t_et_own = const_pool.tile([D, COLS], dt.float32r)
            st0 = stage_pool.tile([D, COLS], dt.float32, tag="st_own")
            nc.sync.dma_start(st0[:, 0:512], et_own[:, 0:512])
            nc.vector.tensor_copy(t_et_own[:, 0:512], st0[:, 0:512])
            t_et_win = const_pool.tile([D, WIN], dt.float32r)
            stw = stage_pool.tile([D, GROUP], dt.float32, tag="st_win")
            nc.gpsimd.dma_start(stw[:], et_win[:])
            for h in range(2):
                nc.vector.tensor_copy(t_et_win[:, h * 1024:(h + 1) * 1024],
                                      stw[:, h * 1024:(h + 1) * 1024])
            t_oh_win = const_pool.tile([NCLS, WIN], dt.bfloat16)
            nc.sync.dma_start(t_oh_win[:], oh_win[:])
            t_ohn_own = const_pool.tile([NCLS, COLS], dt.bfloat16)
            nc.sync.dma_start(t_ohn_own[:], ohn_own[:])
            nc.sync.dma_start(st0[:, 512:], et_own[:, 512:])
            nc.vector.tensor_copy(t_et_own[:, 512:], st0[:, 512:])
            t_et_main = const_pool.tile([D, MAIN], dt.float32r)
            for i in range(NGM):
                st = stage_pool.tile([D, GROUP], dt.float32, tag="st")
                nc.sync.dma_start(st[:], et_main[:, i * GROUP:(i + 1) * GROUP])
                for h in range(2):
                    lo = i * GROUP + h * 1024
                    nc.vector.tensor_copy(t_et_main[:, lo:lo + 1024],
                                          st[:, h * 1024:(h + 1) * 1024])

            s_part = const_pool.tile([128, NCT * NSLOT], dt.float32)
            a_part = const_pool.tile([128, NCT], dt.float32)
            m_raw = const_pool.tile([128, NCT], dt.float32)
            m_sb = const_pool.tile([128, NCT], dt.float32)

            def emit_ln(ct, R):
                # ce-sum: Ln(m_j / Pwin + 1), fused per-column accumulate.
                dump = d_pool.tile([128, WIN], dt.float32, tag="dump")
                nc.scalar.activation(
                    dump[:], R[:], mybir.ActivationFunctionType.Ln,
                    scale=m_sb[:, ct:ct + 1], bias=1.0,
                    accum_out=a_part[:, ct:ct + 1])

            prev = None          # (ct, P) whose Ln is deferred one col-tile
            for ct in range(NCT):
                lhs_et = t_et_own[:, ct * 128:(ct + 1) * 128]
                lhs_oh = t_ohn_own[:, ct * 128:(ct + 1) * 128]

                # ---- window rows first: masked; feed S and the ce sum.
                # Doing these before the main groups starts the DVE S/m
                # chain early so the deferred Ln is never waiting.
                ps = ps_pool.tile([128, GROUP], dt.float32, tag="ps")
                for n in range(NSUB):
                    nc.tensor.matmul(
                        ps[:, n * 512:(n + 1) * 512], lhs_et,
                        t_et_win[:, n * 512:(n + 1) * 512],
                        start=True, stop=False)
                for n in range(NSUB):
                    nc.tensor.matmul(
                        ps[:, n * 512:(n + 1) * 512], lhs_oh,
                        t_oh_win[:, n * 512:(n + 1) * 512],
                        start=False, stop=True)
                # Pwin = exp(l - C*same); its fused accum IS the masked S_win.
                # The Ln input exp(-l + C*same) comes from the DVE reciprocal.
                P = p_pool.tile([128, WIN], dt.float32, tag="P")
                slot = ct * NSLOT + NGM
                nc.scalar.activation(
                    P[:], ps[:], mybir.ActivationFunctionType.Exp,
                    scale=SCALE, accum_out=s_part[:, slot:slot + 1])
                R = r_pool.tile([128, WIN], dt.float32, tag="R")
                nc.vector.reciprocal_approx_fast(out=R[:], in_=P[:])

                # ---- main rows: unmasked, only feed S ----
                for g in range(NGM):
                    r0 = g * GROUP
                    ps = ps_pool.tile([128, GROUP], dt.float32, tag="ps")
                    for n in range(NSUB):
                        nc.tensor.matmul(
                            ps[:, n * 512:(n + 1) * 512], lhs_et,
                            t_et_main[:, r0 + n * 512:r0 + (n + 1) * 512],
                            start=True, stop=True)
                    dump = d_pool.tile([128, GROUP], dt.float32, tag="dump")
                    slot = ct * NSLOT + g
                    nc.scalar.activation(
                        dump[:], ps[:], mybir.ActivationFunctionType.Exp,
                        scale=SCALE, accum_out=s_part[:, slot:slot + 1])

                nc.vector.reduce_sum(
                    m_raw[:, ct:ct + 1],
                    s_part[:, ct * NSLOT:(ct + 1) * NSLOT],
                    axis=mybir.AxisListType.X)
                nc.vector.tensor_scalar_mul(
                    m_sb[:, ct:ct + 1], m_raw[:, ct:ct + 1], MCONST)

                # Defer this tile's Ln so the ACT FIFO can run the next
                # tile's exps while the DVE S/m chain completes.
                if prev is not None:
                    emit_ln(*prev)
                prev = (ct, R)
            emit_ln(*prev)

            nc.sync.dma_start(out[:, 0:NCT * NSLOT], s_part[:])
            nc.sync.dma_start(out[:, NCT * NSLOT:], a_part[:])
    nc.compile()
    return nc


def _get_nc():
    if "nc" not in _cache:
        _cache["nc"] = _build()
    return _cache["nc"]


def _prepare(embeds, labels):
    embeds = np.ascontiguousarray(np.asarray(embeds, dtype=np.float32))
    labels_i = np.asarray(labels).astype(np.int64)
    assert embeds.shape == (B, D)

    # Sort columns (and rows -- it is the same axis) by label so each
    # core's same-label rows are contiguous.
    perm = np.argsort(labels_i, kind="stable")
    lab = labels_i[perm]
    emb = embeds[perm]

    et = np.ascontiguousarray(emb.T)                      # [D, B] f32
    oh = np.zeros((NCLS, B), dtype=ml_dtypes.bfloat16)
    oh[lab, np.arange(B)] = ml_dtypes.bfloat16(1.0)
    ohn = (oh.astype(np.float32) * np.float32(MASKVAL)).astype(ml_dtypes.bfloat16)

    # class start/end in sorted order
    starts = np.searchsorted(lab, np.arange(NCLS), side="left")
    ends = np.searchsorted(lab, np.arange(NCLS), side="right")

    in_maps = []
    windows = []
    for c in range(NCORES):
        lo, hi = c * COLS, (c + 1) * COLS
        r_lo = int(starts[lab[lo]])
        r_hi = int(ends[lab[hi - 1]])
        assert r_hi - r_lo <= WIN, f"window overflow: {r_hi - r_lo}"
        w0 = max(0, min((r_lo // 512) * 512, B - WIN))
        if w0 + WIN < r_hi:                       # need to shift right
            w0 = min(((r_hi + 511) // 512) * 512 - WIN, B - WIN)
        assert w0 <= r_lo and w0 + WIN >= r_hi
        windows.append(w0)
        main_idx = np.r_[0:w0, w0 + WIN:B]
        in_maps.append({
            "et_main": np.ascontiguousarray(et[:, main_idx]),
            "et_win": np.ascontiguousarray(et[:, w0:w0 + WIN]),
            "et_own": np.ascontiguousarray(et[:, lo:hi]),
            "oh_win": np.ascontiguousarray(oh[:, w0:w0 + WIN]),
            "ohn_own": np.ascontiguousarray(ohn[:, lo:hi]),
        })
    return in_maps, lab, emb


def _combine(results, lab, emb):
    NSLOT = NGM + 1
    S = np.empty(B, dtype=np.float64)
    A = np.empty(B, dtype=np.float64)
    for c in range(NCORES):
        o = results[c]["out"].astype(np.float64)
        s = o[:, :NCT * NSLOT].reshape(128, NCT, NSLOT).sum(-1)   # [p, ct]
        a = o[:, NCT * NSLOT:NCT * NSLOT + NCT]                   # [p, ct]
        S[c * COLS:(c + 1) * COLS] = s.T.reshape(-1)              # j = ct*128+p
        A[c * COLS:(c + 1) * COLS] = a.T.reshape(-1)

    counts = np.bincount(lab, minlength=NCLS)
    count_j = counts[lab].astype(np.float64) - 1.0
    l_jj = (emb.astype(np.float64) ** 2).sum(1) * SCALE
    # A_j = sum_{in_numer}(ce+eps) + (ce_jj+eps); u_jj = ce_jj + eps, so
    # numer = A_j - eps*count_j - u_jj.
    u_jj = np.log1p(S * np.exp(EPS - l_jj))
    numer = A - EPS * count_j - u_jj
    loss = (numer / count_j).sum() / B
    return np.asarray(loss, dtype=np.float32)


def kernel(embeds, labels):
    in_maps, lab, emb = _prepare(embeds, labels)
    nc = _get_nc()
    res = run_bass_kernel_spmd(nc, in_maps, list(range(NCORES)))
    return _combine(res.results, lab, emb)


# revision 46
# speedup vs baseline: 1.0231x; 1.0183x over previous
"""EpsSupInfoNCE loss on 8 Trainium2 NeuronCores.

Math (reference): logits = (E @ E.T)/temp;  same[i,j] = (label_i == label_j)
  S_j   = sum_i exp(logits[i,j]) * (1 - same[i,j])     (masked column sums)
  ce_ij = log(exp(-eps) + S_j * exp(-logits[i,j]))     for same-label i != j
  loss  = sum_j (1/count_j) * sum_i ce_ij / B

Strategy: columns are sharded over 8 cores; the HOST sorts all columns by
label first. Core c owns 1024 sorted columns whose same-label rows then
live in ONE contiguous row interval, padded to a 2048-row "window" (max
class size ~130, so 1024 + 2*130 always fits). Rows are passed to each
core pre-split into et_win [D,2048] and et_main [D,6144] (order of rows
is irrelevant for the sums), which keeps the program SPMD-identical
across cores while the actual window offset varies per core.

Per 128-column tile:
  main rows:   logits matmul (fp32r, single PE pass) -> PSUM;
               ACT exp(+l/temp) with fused accum_out -> S partials.
               No label mask needed: no same-label pairs here.
  window rows: logits matmul + one-hot mask matmul (-4.5 -> -C in logit
               units) -> PSUM; ACT exp(l/temp - C*same) -> Pwin, whose
               fused accum IS the masked S_win partial; DVE reciprocal
               R = 1/Pwin; ACT Ln(m_j * R + 1) with accum_out -> A_j
               partials, m_j = S_j*e^(eps-C): equals ce+eps at same-label
               entries, ~1e-17 at different-label ones.
Host: numer_sum_j = A_j - eps*count_j - log1p(S_j e^(eps-l_jj)) (the
diagonal term, which carries its own +eps), then a tiny f64 reduction. Out-of-window same-label
terms do not exist; out-of-window Ln terms are < 1e-13 and are dropped.
"""
import numpy as np
import ml_dtypes
from contextlib import ExitStack

import concourse.bacc as bacc
import concourse.tile as tile
from concourse import mybir
from concourse.bass_utils import run_bass_kernel_spmd

B, D = 8192, 128
NCLS = 100
NCORES = 8
COLS = B // NCORES            # 1024 columns per core
NCT = COLS // 128             # 8 col-tiles per core
WIN = 2048                    # window rows per core
MAIN = B - WIN                # 6144 main rows per core
GROUP = 2048                  # rows per PSUM group (4 banks)
NGM = MAIN // GROUP           # 3 main groups
NSUB = GROUP // 512
LNW = 1536                    # Ln/recip slice: true same-label block size

TEMP = 0.07
EPS = 0.25
SCALE = float(np.float32(1.0) / np.float32(TEMP))   # exp scale (fp32 value)
MASKVAL = -4.5                                      # bf16-exact additive mask
C_USED = 4.5 * SCALE                                # mask size in logit units
MCONST = float(np.exp(EPS - C_USED))                # e^(eps-C)

_cache = {}


def _patch_act_tables():
    """Steer Exp and Ln onto the one table set holding both, so Exp/Ln
    alternation doesn't thrash ACT_TABLE_LOADs. Set ids are indices into
    act_info.json, so keep dict length/order and just hide exp/ln
    elsewhere."""
    import concourse.hw_specs as hw_specs
    from concourse import mybir as _mb
    if getattr(bacc, "_act_tables_patched", False):
        return
    orig = hw_specs.get_activation_tables

    def steer(arch):
        t = orig(arch)
        exp, ln = (_mb.ActivationFunctionType.Exp, _mb.ActivationFunctionType.Ln)
        if "natural_log_exp_and_others" not in t:
            return t
        return {k: (fns if k == "natural_log_exp_and_others"
                    else fns - {exp, ln}) for k, fns in t.items()}

    bacc.get_activation_tables = steer
    bacc._act_tables_patched = True


def _build():
    dt = mybir.dt
    _patch_act_tables()
    nc = bacc.Bacc("TRN2", target_bir_lowering=False, debug=False,
                   num_devices=NCORES)
    et_main = nc.dram_tensor("et_main", [D, MAIN], dt.float32,
                             kind="ExternalInput").ap()
    et_win = nc.dram_tensor("et_win", [D, WIN], dt.float32,
                            kind="ExternalInput").ap()
    et_own = nc.dram_tensor("et_own", [D, COLS], dt.float32,
                            kind="ExternalInput").ap()
    oh_win = nc.dram_tensor("oh_win", [NCLS, WIN], dt.bfloat16,
                            kind="ExternalInput").ap()
    ohn_own = nc.dram_tensor("ohn_own", [NCLS, COLS], dt.bfloat16,
                             kind="ExternalInput").ap()
    NSLOT = NGM + 1                                    # S slots per col-tile
    out = nc.dram_tensor("out", [128, NCT * NSLOT + NCT], dt.float32,
                         kind="ExternalOutput").ap()

    with tile.TileContext(nc) as tc:
        with ExitStack() as ctx:
            const_pool = ctx.enter_context(tc.tile_pool(name="consts", bufs=1))
            p_pool = ctx.enter_context(tc.tile_pool(name="pwin", bufs=2))
            r_pool = ctx.enter_context(tc.tile_pool(name="rbuf", bufs=2))
            d_pool = ctx.enter_context(tc.tile_pool(name="dump", bufs=2))
            stage_pool = ctx.enter_context(tc.tile_pool(name="stage", bufs=2))
            ps_pool = ctx.enter_context(
                tc.tile_pool(name="psum", bufs=2, space="PSUM"))

            # fp32r matmul operands must be rounded by a compute op: DMA to
            # fp32 staging, DVE-copy (rounds) into fp32r tiles. Load order:
            # what the first col-tile needs first.
            # Load order mirrors first-consumption order: ct0's window work
            # needs et_own[:, :128], et_win, oh_win, ohn_own; the main
            # groups then need et_main chunks. Two DMA queues in parallel.
            # ct0's first matmul needs only et_own[:, :128] and et_win chunk
            # 0, so chunk those finely and alternate DMA queues to overlap
            # transfer with the first window matmuls.
            t_et_own = const_pool.tile([D, COLS], dt.float32r)
            st0 = stage_pool.tile([D, COLS], dt.float32, tag="st_own")
            nc.sync.dma_start(st0[:, 0:512], et_own[:, 0:512])
            nc.vector.tensor_copy(t_et_own[:, 0:512], st0[:, 0:512])
            t_et_win = const_pool.tile([D, WIN], dt.float32r)
            stw = stage_pool.tile([D, GROUP], dt.float32, tag="st_win")
            nc.gpsimd.dma_start(stw[:], et_win[:])
            for h in range(2):
                nc.vector.tensor_copy(t_et_win[:, h * 1024:(h + 1) * 1024],
                                      stw[:, h * 1024:(h + 1) * 1024])
            t_oh_win = const_pool.tile([NCLS, WIN], dt.bfloat16)
            nc.sync.dma_start(t_oh_win[:], oh_win[:])
            t_ohn_own = const_pool.tile([NCLS, COLS], dt.bfloat16)
            nc.sync.dma_start(t_ohn_own[:], ohn_own[:])
            nc.sync.dma_start(st0[:, 512:], et_own[:, 512:])
            nc.vector.tensor_copy(t_et_own[:, 512:], st0[:, 512:])
            t_et_main = const_pool.tile([D, MAIN], dt.float32r)
            for i in range(NGM):
                st = stage_pool.tile([D, GROUP], dt.float32, tag="st")
                nc.sync.dma_start(st[:], et_main[:, i * GROUP:(i + 1) * GROUP])
                for h in range(2):
                    lo = i * GROUP + h * 1024
                    nc.vector.tensor_copy(t_et_main[:, lo:lo + 1024],
                                          st[:, h * 1024:(h + 1) * 1024])

            s_part = const_pool.tile([128, NCT * NSLOT], dt.float32)
            a_part = const_pool.tile([128, NCT], dt.float32)
            m_raw = const_pool.tile([128, NCT], dt.float32)
            m_sb = const_pool.tile([128, NCT], dt.float32)

            def emit_ln(ct, R):
                # ce-sum: Ln(m_j / Pwin + 1), fused per-column accumulate.
                dump = d_pool.tile([128, LNW], dt.float32, tag="dump2")
                nc.scalar.activation(
                    dump[:], R[:], mybir.ActivationFunctionType.Ln,
                    scale=m_sb[:, ct:ct + 1], bias=1.0,
                    accum_out=a_part[:, ct:ct + 1])

            prev = None          # (ct, P) whose Ln is deferred one col-tile
            for ct in range(NCT):
                lhs_et = t_et_own[:, ct * 128:(ct + 1) * 128]
                lhs_oh = t_ohn_own[:, ct * 128:(ct + 1) * 128]

                # ---- window rows first: masked; feed S and the ce sum.
                # Doing these before the main groups starts the DVE S/m
                # chain early so the deferred Ln is never waiting.
                ps = ps_pool.tile([128, GROUP], dt.float32, tag="ps")
                for n in range(NSUB):
                    nc.tensor.matmul(
                        ps[:, n * 512:(n + 1) * 512], lhs_et,
                        t_et_win[:, n * 512:(n + 1) * 512],
                        start=True, stop=False)
                for n in range(NSUB):
                    nc.tensor.matmul(
                        ps[:, n * 512:(n + 1) * 512], lhs_oh,
                        t_oh_win[:, n * 512:(n + 1) * 512],
                        start=False, stop=True)
                # Pwin = exp(l - C*same); its fused accum IS the masked S_win.
                # The Ln input exp(-l + C*same) comes from the DVE reciprocal.
                P = p_pool.tile([128, WIN], dt.float32, tag="P")
                slot = ct * NSLOT + NGM
                nc.scalar.activation(
                    P[:], ps[:], mybir.ActivationFunctionType.Exp,
                    scale=SCALE, accum_out=s_part[:, slot:slot + 1])
                # Same-label rows sit at window offset 0 (host layout), so
                # the numerator path only needs the first LNW rows.
                R = r_pool.tile([128, LNW], dt.float32, tag="R")
                nc.vector.reciprocal_approx_fast(out=R[:], in_=P[:, 0:LNW])

                # ---- main rows: unmasked, only feed S ----
                for g in range(NGM):
                    r0 = g * GROUP
                    ps = ps_pool.tile([128, GROUP], dt.float32, tag="ps")
                    for n in range(NSUB):
                        nc.tensor.matmul(
                            ps[:, n * 512:(n + 1) * 512], lhs_et,
                            t_et_main[:, r0 + n * 512:r0 + (n + 1) * 512],
                            start=True, stop=True)
                    dump = d_pool.tile([128, GROUP], dt.float32, tag="dump")
                    slot = ct * NSLOT + g
                    nc.scalar.activation(
                        dump[:], ps[:], mybir.ActivationFunctionType.Exp,
                        scale=SCALE, accum_out=s_part[:, slot:slot + 1])

                nc.vector.reduce_sum(
                    m_raw[:, ct:ct + 1],
                    s_part[:, ct * NSLOT:(ct + 1) * NSLOT],
                    axis=mybir.AxisListType.X)
                nc.vector.tensor_scalar_mul(
                    m_sb[:, ct:ct + 1], m_raw[:, ct:ct + 1], MCONST)

                # Defer this tile's Ln so the ACT FIFO can run the next
                # tile's exps while the DVE S/m chain completes.
                if prev is not None:
                    emit_ln(*prev)
                prev = (ct, R)
            emit_ln(*prev)

            nc.sync.dma_start(out[:, 0:NCT * NSLOT], s_part[:])
            nc.sync.dma_start(out[:, NCT * NSLOT:], a_part[:])
    nc.compile()
    return nc


def _get_nc():
    if "nc" not in _cache:
        _cache["nc"] = _build()
    return _cache["nc"]


def _prepare(embeds, labels):
    embeds = np.ascontiguousarray(np.asarray(embeds, dtype=np.float32))
    labels_i = np.asarray(labels).astype(np.int64)
    assert embeds.shape == (B, D)

    # Sort columns (and rows -- it is the same axis) by label so each
    # core's same-label rows are contiguous.
    perm = np.argsort(labels_i, kind="stable")
    lab = labels_i[perm]
    emb = embeds[perm]

    et = np.ascontiguousarray(emb.T)                      # [D, B] f32
    oh = np.zeros((NCLS, B), dtype=ml_dtypes.bfloat16)
    oh[lab, np.arange(B)] = ml_dtypes.bfloat16(1.0)
    ohn = (oh.astype(np.float32) * np.float32(MASKVAL)).astype(ml_dtypes.bfloat16)

    # class start/end in sorted order
    starts = np.searchsorted(lab, np.arange(NCLS), side="left")
    ends = np.searchsorted(lab, np.arange(NCLS), side="right")

    in_maps = []

    for c in range(NCORES):
        lo, hi = c * COLS, (c + 1) * COLS
        r_lo = int(starts[lab[lo]])
        r_hi = int(ends[lab[hi - 1]])
        span = r_hi - r_lo
        assert span <= LNW, f"window overflow: {span}"
        fill = WIN - span
        after = np.arange(r_hi, min(B, r_hi + fill))
        need = fill - len(after)
        before = np.arange(r_lo - need, r_lo) if need > 0 else np.arange(0)
        win_rows = np.concatenate([np.arange(r_lo, r_hi), after, before])
        assert len(win_rows) == WIN
        main_mask = np.ones(B, dtype=bool)
        main_mask[win_rows] = False
        main_idx = np.nonzero(main_mask)[0]
        in_maps.append({
            "et_main": np.ascontiguousarray(et[:, main_idx]),
            "et_win": np.ascontiguousarray(et[:, win_rows]),
            "et_own": np.ascontiguousarray(et[:, lo:hi]),
            "oh_win": np.ascontiguousarray(oh[:, win_rows]),
            "ohn_own": np.ascontiguousarray(ohn[:, lo:hi]),
        })
    return in_maps, lab, emb


def _combine(results, lab, emb):
    NSLOT = NGM + 1
    S = np.empty(B, dtype=np.float64)
    A = np.empty(B, dtype=np.float64)
    for c in range(NCORES):
        o = results[c]["out"].astype(np.float64)
        s = o[:, :NCT * NSLOT].reshape(128, NCT, NSLOT).sum(-1)   # [p, ct]
        a = o[:, NCT * NSLOT:NCT * NSLOT + NCT]                   # [p, ct]
        S[c * COLS:(c + 1) * COLS] = s.T.reshape(-1)              # j = ct*128+p
        A[c * COLS:(c + 1) * COLS] = a.T.reshape(-1)

    counts = np.bincount(lab, minlength=NCLS)
    count_j = counts[lab].astype(np.float64) - 1.0
    l_jj = (emb.astype(np.float64) ** 2).sum(1) * SCALE
    # A_j = sum_{in_numer}(ce+eps) + (ce_jj+eps); u_jj = ce_jj + eps, so
    # numer = A_j - eps*count_j - u_jj.
    u_jj = np.log1p(S * np.exp(EPS - l_jj))
    numer = A - EPS * count_j - u_jj
    loss = (numer / count_j).sum() / B
    return np.asarray(loss, dtype=np.float32)


def kernel(embeds, labels):
    in_maps, lab, emb = _prepare(embeds, labels)
    nc = _get_nc()
    res = run_bass_kernel_spmd(nc, in_maps, list(range(NCORES)))
    return _combine(res.results, lab, emb)
